# revision 33
# baseline (speedup 1.0000x reference)
"""Multi-head self-attention (B=2, T=2048, D=1024, 16 heads) on 8 TRN2 cores.

Sharding: core c = (b, g) with b = c // 4 (batch), g = c % 4 (head group of 4).
Each core computes q/k/v projections for its 4 heads, causal softmax
attention, and a partial output projection (its 256 columns of the
concat-head dim against Wo). Host sums the 4 partials per batch and adds bo.

v3: same fully-interleaved pass as v2 (attention chunk pipeline with
projection / output-projection fillers), plus:
  - token-sliced HT input DMA across 3 queues so the bootstrap
    projections start at ~25% of the HT fill instead of 100%;
  - compact vt tiles [128, 260] (4 heads x 65: 64 v dims + a ones
    column that makes AV emit softmax denominators) -- no zero padding,
    no gpsimd memsets;
  - the final block's softmax normalization uses an ACT-table
    reciprocal on the PSUM denominator row + a PE ones-matmul
    partition-broadcast instead of the DRAM-bounce + gpsimd path,
    cutting the serial tail;
  - per-half output tiles so the two DMA halves of each O row chunk
    are independent.

Per-core layout:
  qT/kT [128, 2048] bf16: rows = 2 heads x 64 dims, cols = tokens.
  vt[t] [128, 260] bf16: rows = 128 key tokens of chunk t, cols =
     4 heads x 65 (64 v dims + a 1.0 column).
  attT [128, 2048] bf16 per head pair: normalized A^T V rows.
  O [2048, 1024] bf16 partial output, summed on host in f32.
"""

import ml_dtypes
import numpy as np

import concourse.bass as bass
import concourse.tile as tile
from concourse import bacc, mybir
from concourse import bass_utils
from contextlib import ExitStack

F32 = mybir.dt.float32
BF16 = mybir.dt.bfloat16
ATT = BF16
AF = mybir.ActivationFunctionType
OP = mybir.AluOpType

B, T, D = 2, 2048, 1024
NH, DH = 16, 64
HPC = 4            # heads per core
GD = HPC * DH      # 256, group dim
GV = HPC * (DH + 1)  # 260, v tile width (compact, 65 per head)
NKD = D // 128     # 8 K-chunks for projections
NT = T // 128      # 16 token chunks
NJ = T // 512      # 4 query blocks
N_WARM = 12        # HAM clock-ramp warmup matmuls

_NC_CACHE = {}


def build():
    if "nc" in _NC_CACHE:
        return _NC_CACHE["nc"]
    nc = bacc.Bacc("TRN2", target_bir_lowering=False, debug=False, num_devices=8)

    HT = nc.dram_tensor("HT", [D, T], BF16, kind="ExternalInput").ap()
    # weights pre-permuted on the host to the SBUF layout (m-major, then
    # k-chunk-major) so input DMAs are plain streams, not slow gathers --
    # and the m=1 half can be deferred past the bootstrap-critical bytes
    WqR = nc.dram_tensor("WqR", [128, NKD * GD], BF16, kind="ExternalInput").ap()
    WkR = nc.dram_tensor("WkR", [128, NKD * GD], BF16, kind="ExternalInput").ap()
    WvR = nc.dram_tensor("WvR", [128, NKD * GV], BF16, kind="ExternalInput").ap()
    WoS = nc.dram_tensor("WoS", [GD, D], BF16, kind="ExternalInput").ap()
    bqc = nc.dram_tensor("bqc", [128, 2], F32, kind="ExternalInput").ap()
    bkc = nc.dram_tensor("bkc", [128, 2], F32, kind="ExternalInput").ap()
    bvS = nc.dram_tensor("bvS", [1, GV], F32, kind="ExternalInput").ap()
    kpm = nc.dram_tensor("kpm", [128, NT], F32, kind="ExternalInput").ap()
    O = nc.dram_tensor("O", [T, D], BF16, kind="ExternalOutput").ap()
    zd = nc.dram_tensor("zd", [8, 1024], F32, kind="Internal").ap()
    rd = nc.dram_tensor("rd", [8, 1024], F32, kind="Internal").ap()

    with tile.TileContext(nc) as tc, ExitStack() as octx:
        cpool = octx.enter_context(tc.tile_pool(name="const", bufs=1))
        keep = octx.enter_context(tc.tile_pool(name="keep", bufs=1))
        work = octx.enter_context(tc.tile_pool(name="work", bufs=1))
        ps_sc = octx.enter_context(tc.tile_pool(name="ps_sc", bufs=1, space="PSUM"))
        ps_at = octx.enter_context(tc.tile_pool(name="ps_at", bufs=1, space="PSUM"))
        ps_sm = octx.enter_context(tc.tile_pool(name="ps_sm", bufs=1, space="PSUM"))

        # ---- constants ----
        bq_sb = cpool.tile([128, 2], F32, name="bq_sb", tag="bq_sb")
        bk_sb = cpool.tile([128, 2], F32, name="bk_sb", tag="bk_sb")
        bv_sb = cpool.tile([1, GV], F32, name="bv_sb", tag="bv_sb")
        kpm_sb = cpool.tile([128, NT], F32, name="kpm_sb", tag="kpm_sb")

        # ---- long-lived activations ----
        qT = [keep.tile([128, T], ATT, name=f"qT{m}", tag=f"qT{m}") for m in range(2)]
        kT = [keep.tile([128, T], ATT, name=f"kT{m}", tag=f"kT{m}") for m in range(2)]
        vt = [keep.tile([128, GV], ATT, name=f"vt{t}", tag=f"vt{t}") for t in range(NT)]
        attT = [keep.tile([128, T], ATT, name=f"attT{m}", tag=f"attT{m}") for m in range(2)]
        wo_r = [keep.tile([128, D], ATT, name=f"wo{i}", tag=f"wo{i}") for i in range(2)]

        # input H^T, token-sliced: quarters 0/1 as [128,512] tiles, back
        # half as [128,1024] tiles, so projections can start at ~25% of
        # the HT fill.
        ht_q0 = [work.tile([128, 512], BF16, name=f"htq0_{k}", tag=f"htq0_{k}") for k in range(NKD)]
        ht_q1 = [work.tile([128, 512], BF16, name=f"htq1_{k}", tag=f"htq1_{k}") for k in range(NKD)]
        ht_h1 = [work.tile([128, 1024], BF16, name=f"hth1_{k}", tag=f"hth1_{k}") for k in range(NKD)]

        def ht_blk(k, n):
            # [128, 512] slice of H^T k-chunk covering tokens n*512:(n+1)*512
            if n == 0:
                return ht_q0[k][:]
            if n == 1:
                return ht_q1[k][:]
            return ht_h1[k][:, (n - 2) * 512:(n - 1) * 512]

        def ht_tok(k, t):
            # [128, 128] slice covering token chunk t
            n, o = divmod(t, 4)
            return ht_blk(k, n)[:, o * 128:(o + 1) * 128]

        wq_r = work.tile([128, NKD * GD], BF16, name="wq_r", tag="wq_r")
        wk_r = work.tile([128, NKD * GD], BF16, name="wk_r", tag="wk_r")
        wv_r = work.tile([128, NKD * GV], BF16, name="wv_r", tag="wv_r")

        # ---- input DMA, spread across queues ----
        # vector queue: warmup memset; gpsimd: tiny per-core constants
        warm = cpool.tile([128, 512], BF16, name="warm", tag="warm")
        nc.vector.memset(warm[:], 0.0)
        nc.gpsimd.dma_start(bv_sb[:], bvS[:])
        nc.gpsimd.dma_start(bq_sb[:], bqc[:])
        nc.gpsimd.dma_start(bk_sb[:], bkc[:])
        nc.gpsimd.dma_start(kpm_sb[:], kpm[:])
        # Transfers are sequenced by when the data is needed: the DMA rings
        # drain queues in issue order, and everything in flight shares the
        # ~358 GB/s HBM pipe -- so not-yet-needed bytes must queue strictly
        # behind the bootstrap-critical set (m=0 weight halves + wv + HT
        # first quarter ~= 2 MB). m=1 weight halves, the HT back half, and
        # Wo are deferred to their consumption order.
        HW = NKD * 128  # 1024: one m-half of wq/wk
        # scalar queue: bootstrap weights, half the first HT quarter,
        # then the HT back half and the m=1 weight halves
        nc.scalar.dma_start(wq_r[:, 0:HW], WqR[:, 0:HW])
        nc.scalar.dma_start(wk_r[:, 0:HW], WkR[:, 0:HW])
        nc.scalar.dma_start(wv_r[:], WvR[:])
        for k in range(4, NKD):
            nc.scalar.dma_start(ht_q0[k][:], HT[k * 128:(k + 1) * 128, 0:512])
        for k in range(NKD):
            nc.scalar.dma_start(ht_h1[k][:], HT[k * 128:(k + 1) * 128, 1024:2048])
        nc.scalar.dma_start(wq_r[:, HW:2 * HW], WqR[:, HW:2 * HW])
        nc.scalar.dma_start(wk_r[:, HW:2 * HW], WkR[:, HW:2 * HW])
        # sync queue: rest of the first quarter, the second quarter, then Wo
        for k in range(4):
            nc.sync.dma_start(ht_q0[k][:], HT[k * 128:(k + 1) * 128, 0:512])
        for k in range(NKD):
            nc.sync.dma_start(ht_q1[k][:], HT[k * 128:(k + 1) * 128, 512:1024])
        for i in range(2):
            nc.sync.dma_start(wo_r[i][:], WoS[i * 128:(i + 1) * 128, :])
        # gpsimd queue: masks/constants, bv broadcast (stays off the HBM
        # critical path; later it only issues output DMAs)
        tri = cpool.tile([128, 128], ATT, name="tri", tag="tri")
        nc.gpsimd.memset(tri[:], 1.0)
        nc.gpsimd.affine_select(
            out=tri[:], in_=tri[:], compare_op=OP.is_ge, fill=0.0,
            base=0, pattern=[[1, 128]], channel_multiplier=-1,
        )
        # bv broadcast across partitions: [128, GV]
        bvrow = cpool.tile([128, GV], F32, name="bvrow", tag="bvrow")
        nc.gpsimd.partition_broadcast(bvrow[:], bv_sb[:])

        # HAM warmup: keep the PE array busy during the input-DMA fill so
        # the clock gate reaches K=8/8 before real matmuls start (zero
        # data, the results are never read).
        for _ in range(N_WARM):
            wp = ps_sm.tile([128, 512], F32, name="wp", tag="smps", bufs=2)
            nc.tensor.matmul(wp[:], warm[:, 0:128], warm[:], start=True, stop=True)

        # ---- filler work units (PE work injected between attention chunks) ----
        def proj_qk(w_r, dest, bias_sb, m, n, c0=0, c1=512):
            # dest[m][:, n*512+c0 : n*512+c1] = sum_k W_k[:, m-block].T @ ht_k + bias
            w = c1 - c0
            ps = ps_sm.tile([128, 512], F32, name="pp", tag="smps", bufs=2)
            for k in range(NKD):
                nc.tensor.matmul(
                    ps[:, 0:w],
                    w_r[:, (m * NKD + k) * 128:(m * NKD + k) * 128 + 128],
                    ht_blk(k, n)[:, c0:c1],
                    start=(k == 0), stop=(k == NKD - 1),
                )
            # bias add folded into the PSUM->SBUF copy (DVE, per-partition scalar)
            nc.vector.tensor_scalar_add(
                dest[m][:, n * 512 + c0:n * 512 + c1], ps[:, 0:w],
                bias_sb[:, m:m + 1]
            )

        def proj_v(t):
            # vt[t] per-head blocks = (sum_k ht_k_t.T @ WvS_k + bv) * kpm
            vp = ps_sm.tile([128, 512], F32, name="vp", tag="smps", bufs=2)
            for k in range(NKD):
                nc.tensor.matmul(
                    vp[:, 0:GV],
                    ht_tok(k, t),
                    wv_r[:, k * GV:(k + 1) * GV],
                    start=(k == 0), stop=(k == NKD - 1),
                )
            nc.vector.tensor_tensor(
                vt[t][:], vp[:, 0:GV], bvrow[:], op=OP.add,
            )
            nc.vector.tensor_scalar_mul(vt[t][:], vt[t][:], kpm_sb[:, t:t + 1])

        def out_half(t, n):
            # O[t-chunk, n-half] = sum_hp attT[hp][:, t-chunk].T @ WoS[hp][:, n-half]
            ot = work.tile([128, 512], BF16, name="ot", tag="ot", bufs=4)
            op = ps_sm.tile([128, 512], F32, name="op", tag="smps", bufs=2)
            for hp in range(2):
                nc.tensor.matmul(
                    op[:],
                    attT[hp][:, t * 128:(t + 1) * 128],
                    wo_r[hp][:, n * 512:(n + 1) * 512],
                    start=(hp == 0), stop=(hp == 1),
                )
            # PSUM->SBUF copy on the scalar engine (Identity shares the Exp
            # ACT table): frees the DVE, which runs the normalize multiplies.
            # Output DMA issues from the gpsimd queue (idle mid-kernel).
            nc.scalar.activation(ot[:], op[:], AF.Identity)
            nc.gpsimd.dma_start(O[t * 128:(t + 1) * 128, n * 512:(n + 1) * 512],
                                ot[:])

        fillers = []

        def pop_filler():
            if fillers:
                fillers.pop(0)()

        # ---- attention block machinery ----
        def normalize(hp, J, at):
            # zau: unnormalized A^T V rows (0:64) + denominator row (64)
            zaus = []
            for hh in range(2):
                zau = work.tile([65, 512], F32, name="zau", tag="zau", bufs=4)
                nc.vector.tensor_copy(zau[:], at[hh][0:65, :])
                zaus.append(zau)
            bi = hp * 4 + J
            # exact reciprocal on a partition-packed [128, 8] tile: bounce the
            # two denominator rows through DRAM (engines cannot cross
            # partitions; DMA can). 8 elem/lane keeps the iterative divide
            # at ~130 ns instead of 4.3 us on a [1, 512] row.
            for hh in range(2):
                nc.sync.dma_start(zd[bi:bi + 1, hh * 512:(hh + 1) * 512],
                                    zaus[hh][64:65, :])
            zp = work.tile([128, 8], F32, name="zp", tag="zp", bufs=2)
            nc.sync.dma_start(
                zp[:], zd[bi:bi + 1, :].rearrange("p (a b) -> (p a) b", b=8)
            )
            rp = work.tile([128, 8], F32, name="rp", tag="rp", bufs=2)
            nc.vector.reciprocal(rp[:], zp[:])
            nc.sync.dma_start(
                rd[bi:bi + 1, :].rearrange("p (a b) -> (p a) b", b=8), rp[:]
            )
            for hh in range(2):
                zau = zaus[hh]
                # partition-broadcast 1/z straight out of DRAM with a
                # 0-stride DMA read (frees gpsimd, one fewer serial hop)
                rb = work.tile([64, 512], F32, name="rb", tag="rb", bufs=4)
                nc.sync.dma_start(
                    rb[:],
                    rd[bi:bi + 1, hh * 512:(hh + 1) * 512]
                    .squeeze(0).partition_broadcast(64),
                )
                nc.vector.tensor_tensor(
                    attT[hp][hh * 64:(hh + 1) * 64, J * 512:(J + 1) * 512],
                    zau[0:64, :],
                    rb[:],
                    op=OP.mult,
                )

        def block(J, hp):
            at = [
                ps_at.tile([128, 512], F32, name=f"at{hh}", tag="av", bufs=2)
                for hh in range(2)
            ]
            # diagonal chunk first (full width, opens PSUM accumulation),
            # then off-diagonals, then narrow diagonals.
            kcs = [4 * J] + list(range(4 * J)) + [4 * J + i for i in range(1, 4)]

            def issue_sc_exp(kc):
                off = max(0, 128 * (kc - 4 * J))
                w = 512 - off
                sc = ps_sc.tile([128, 1024], F32, name="sc", tag="sc", bufs=2)
                for hh in range(2):
                    nc.tensor.matmul(
                        sc[:, hh * 512:hh * 512 + w],
                        kT[hp][hh * 64:(hh + 1) * 64, kc * 128:(kc + 1) * 128],
                        qT[hp][hh * 64:(hh + 1) * 64, J * 512 + off:(J + 1) * 512],
                        start=True, stop=True,
                        tile_position=(hh * 64, 0),
                    )
                ex = work.tile([128, 1024], ATT, name="ex", tag="ex", bufs=8)
                nc.scalar.activation(
                    ex[:].rearrange("p (h c) -> p h c", c=512)[:, :, 0:w],
                    sc[:].rearrange("p (h c) -> p h c", c=512)[:, :, 0:w],
                    AF.Exp, scale=0.125,
                )
                if off or kc == 4 * J:
                    for hh in range(2):
                        nc.vector.tensor_tensor(
                            ex[:, hh * 512:hh * 512 + 128],
                            ex[:, hh * 512:hh * 512 + 128],
                            tri[:],
                            op=OP.mult,
                        )
                return ex

            def issue_av(kc, ex, first, last):
                off = max(0, 128 * (kc - 4 * J))
                w = 512 - off
                for hh in range(2):
                    h = 2 * hp + hh
                    nc.tensor.matmul(
                        at[hh][0:65, off:512],
                        vt[kc][:, h * 65:(h + 1) * 65],
                        ex[:, hh * 512:hh * 512 + w],
                        start=first, stop=last,
                    )

            prev = None
            for ti, kc in enumerate(kcs):
                ex = issue_sc_exp(kc)
                pop_filler()
                if prev is not None:
                    issue_av(prev[0], prev[1], first=(prev[2] == 0), last=False)
                prev = (kc, ex, ti)
            issue_av(prev[0], prev[1], first=(prev[2] == 0), last=True)
            normalize(hp, J, at)

        # ---- bootstrap projections for the first diagonal chunk ----
        # minimal set for the first score/AV chunk: full q block, the first
        # 128 key columns, vt[0]; everything else becomes fillers so the PE
        # never idles (idling drops the HAM clock gate to K=4).
        proj_qk(wq_r, qT, bq_sb, 0, 0)
        proj_qk(wk_r, kT, bk_sb, 0, 0, 0, 128)
        proj_v(0)

        def qk(hp_, n_):
            fillers.append(lambda: proj_qk(wq_r, qT, bq_sb, hp_, n_))
            fillers.append(lambda: proj_qk(wk_r, kT, bk_sb, hp_, n_))

        # filler schedule, matched to each block's pop budget (block(J,hp)
        # pops 4J+4 fillers; two extra boundary pops follow block(0,0)) and
        # to the token-sliced HT DMA arrival order. Each proj must be popped
        # no later than its first consumer chunk in the block pipeline.
        fillers.append(lambda: proj_qk(wk_r, kT, bk_sb, 0, 0, 128, 512))
        for t in (1, 2, 3):
            fillers.append(lambda t=t: proj_v(t))
        qk(0, 1)                                     # boundary pops
        for t in (4, 5):
            fillers.append(lambda t=t: proj_v(t))
        qk(0, 2)
        for t in (6, 7):
            fillers.append(lambda t=t: proj_v(t))
        qk(0, 3)
        for t in (8, 9, 10, 11):
            fillers.append(lambda t=t: proj_v(t))
        for n in reversed(range(4)):
            qk(1, n)
        for t in (12, 13, 14, 15):
            fillers.append(lambda t=t: proj_v(t))

        # ---- main pass ----
        # hp1 runs J descending: the big J=3 block comes first (making its
        # output-projection fillers available early) and the small J=0 block
        # lands last, shortening the final normalize->out tail.
        for hp, Js in ((0, range(NJ)), (1, reversed(range(NJ)))):
            for J in Js:
                block(J, hp)
                if hp == 0 and J == 0:
                    pop_filler()
                    pop_filler()
                if hp == 1:
                    # attT for both head pairs at J is now final
                    for t in range(4 * J, 4 * J + 4):
                        for n in range(2):
                            fillers.append(lambda t=t, n=n: out_half(t, n))
        while fillers:
            fillers.pop(0)()

    nc.compile()
    _NC_CACHE["nc"] = nc
    return nc


def _prep_core_inputs(H, key_padding_mask, Wq, bq, Wk, bk, Wv, bv, Wo, bo):
    keep = 1.0 - np.asarray(key_padding_mask, dtype=np.float32)  # [B, T]
    bf = ml_dtypes.bfloat16
    in_maps = []
    def to_sbuf_layout(WT):
        # [D, G] -> [128, NKD*G]: row p holds k-chunk-major slices
        G = WT.shape[1]
        return np.ascontiguousarray(
            WT.reshape(NKD, 128, G).transpose(1, 0, 2).reshape(128, NKD * G))

    def to_sbuf_layout_m(WT):
        # [D, 256] -> [128, 2*NKD*128]: m-half-major, then k-chunk-major
        return np.ascontiguousarray(
            WT.reshape(NKD, 128, 2, 128).transpose(1, 2, 0, 3)
            .reshape(128, 2 * NKD * 128))

    for c in range(8):
        b, g = divmod(c, 4)
        sl = slice(g * GD, (g + 1) * GD)
        WvT = Wv[sl].T  # [D, GD]
        WvS = np.zeros((D, GV), dtype=np.float32)
        bvS = np.zeros((1, GV), dtype=np.float32)
        for h in range(HPC):
            WvS[:, h * 65:h * 65 + 64] = WvT[:, h * 64:(h + 1) * 64]
            bvS[0, h * 65:h * 65 + 64] = bv[sl][h * 64:(h + 1) * 64]
            bvS[0, h * 65 + 64] = 1.0
        in_maps.append({
            "HT": np.ascontiguousarray(H[b].T).astype(bf),
            "WqR": to_sbuf_layout_m(Wq[sl].T).astype(bf),
            "WkR": to_sbuf_layout_m(Wk[sl].T).astype(bf),
            "WvR": to_sbuf_layout(WvS).astype(bf),
            "WoS": np.ascontiguousarray(Wo[:, sl].T).astype(bf),
            "bqc": np.ascontiguousarray(bq[sl].reshape(2, 128).T.astype(np.float32)),
            "bkc": np.ascontiguousarray(bk[sl].reshape(2, 128).T.astype(np.float32)),
            "bvS": bvS,
            "kpm": np.ascontiguousarray(keep[b].reshape(NT, 128).T),
        })
    return in_maps


def kernel(H, key_padding_mask, Wq, bq, Wk, bk, Wv, bv, Wo, bo, _run_kwargs=None):
    H = np.asarray(H, dtype=np.float32)
    Wq = np.asarray(Wq, dtype=np.float32)
    Wk = np.asarray(Wk, dtype=np.float32)
    Wv = np.asarray(Wv, dtype=np.float32)
    Wo = np.asarray(Wo, dtype=np.float32)
    bq = np.asarray(bq, dtype=np.float32)
    bk = np.asarray(bk, dtype=np.float32)
    bv = np.asarray(bv, dtype=np.float32)
    bo = np.asarray(bo, dtype=np.float32)

    nc = build()
    in_maps = _prep_core_inputs(H, key_padding_mask, Wq, bq, Wk, bk, Wv, bv, Wo, bo)
    res = bass_utils.run_bass_kernel_spmd(
        nc, in_maps, core_ids=list(range(8)), **(_run_kwargs or {})
    )
    out = np.zeros((B, T, D), dtype=np.float32)
    for c in range(8):
        out[c // 4] += res.results[c]["O"].astype(np.float32)
    out += bo
    if _run_kwargs:
        kernel.last_result = res
    return out


# revision 36
# speedup vs baseline: 1.0068x; 1.0068x over previous
"""Multi-head self-attention (B=2, T=2048, D=1024, 16 heads) on 8 TRN2 cores.

Sharding: core c = (b, g) with b = c // 4 (batch), g = c % 4 (head group of 4).
Each core computes q/k/v projections for its 4 heads, causal softmax
attention, and a partial output projection (its 256 columns of the
concat-head dim against Wo). Host sums the 4 partials per batch and adds bo.

v3: same fully-interleaved pass as v2 (attention chunk pipeline with
projection / output-projection fillers), plus:
  - token-sliced HT input DMA across 3 queues so the bootstrap
    projections start at ~25% of the HT fill instead of 100%;
  - compact vt tiles [128, 260] (4 heads x 65: 64 v dims + a ones
    column that makes AV emit softmax denominators) -- no zero padding,
    no gpsimd memsets;
  - the final block's softmax normalization uses an ACT-table
    reciprocal on the PSUM denominator row + a PE ones-matmul
    partition-broadcast instead of the DRAM-bounce + gpsimd path,
    cutting the serial tail;
  - per-half output tiles so the two DMA halves of each O row chunk
    are independent.

Per-core layout:
  qT/kT [128, 2048] bf16: rows = 2 heads x 64 dims, cols = tokens.
  vt[t] [128, 260] bf16: rows = 128 key tokens of chunk t, cols =
     4 heads x 65 (64 v dims + a 1.0 column).
  attT [128, 2048] bf16 per head pair: normalized A^T V rows.
  O [2048, 1024] bf16 partial output, summed on host in f32.
"""

import ml_dtypes
import numpy as np

import concourse.bass as bass
import concourse.tile as tile
from concourse import bacc, mybir
from concourse import bass_utils
from contextlib import ExitStack

F32 = mybir.dt.float32
BF16 = mybir.dt.bfloat16
ATT = BF16
AF = mybir.ActivationFunctionType
OP = mybir.AluOpType

B, T, D = 2, 2048, 1024
NH, DH = 16, 64
HPC = 4            # heads per core
GD = HPC * DH      # 256, group dim
GV = HPC * (DH + 1)  # 260, v tile width (compact, 65 per head)
NKD = D // 128     # 8 K-chunks for projections
NT = T // 128      # 16 token chunks
NJ = T // 512      # 4 query blocks
N_WARM = 12        # HAM clock-ramp warmup matmuls

_NC_CACHE = {}


def build():
    if "nc" in _NC_CACHE:
        return _NC_CACHE["nc"]
    nc = bacc.Bacc("TRN2", target_bir_lowering=False, debug=False, num_devices=8)

    HT = nc.dram_tensor("HT", [D, T], BF16, kind="ExternalInput").ap()
    # weights pre-permuted on the host to the SBUF layout (m-major, then
    # k-chunk-major) so input DMAs are plain streams, not slow gathers --
    # and the m=1 half can be deferred past the bootstrap-critical bytes
    WqR = nc.dram_tensor("WqR", [128, NKD * GD], BF16, kind="ExternalInput").ap()
    WkR = nc.dram_tensor("WkR", [128, NKD * GD], BF16, kind="ExternalInput").ap()
    WvR = nc.dram_tensor("WvR", [128, NKD * GV], BF16, kind="ExternalInput").ap()
    WoS = nc.dram_tensor("WoS", [GD, D], BF16, kind="ExternalInput").ap()
    bqc = nc.dram_tensor("bqc", [128, 2], F32, kind="ExternalInput").ap()
    bkc = nc.dram_tensor("bkc", [128, 2], F32, kind="ExternalInput").ap()
    bvS = nc.dram_tensor("bvS", [1, GV], F32, kind="ExternalInput").ap()
    kpm = nc.dram_tensor("kpm", [128, NT], F32, kind="ExternalInput").ap()
    O = nc.dram_tensor("O", [T, D], BF16, kind="ExternalOutput").ap()
    zd = nc.dram_tensor("zd", [8, 1024], F32, kind="Internal").ap()
    rd = nc.dram_tensor("rd", [8, 1024], F32, kind="Internal").ap()

    with tile.TileContext(nc) as tc, ExitStack() as octx:
        cpool = octx.enter_context(tc.tile_pool(name="const", bufs=1))
        keep = octx.enter_context(tc.tile_pool(name="keep", bufs=1))
        work = octx.enter_context(tc.tile_pool(name="work", bufs=1))
        ps_sc = octx.enter_context(tc.tile_pool(name="ps_sc", bufs=1, space="PSUM"))
        ps_at = octx.enter_context(tc.tile_pool(name="ps_at", bufs=1, space="PSUM"))
        ps_sm = octx.enter_context(tc.tile_pool(name="ps_sm", bufs=1, space="PSUM"))

        # ---- constants ----
        bq_sb = cpool.tile([128, 2], F32, name="bq_sb", tag="bq_sb")
        bk_sb = cpool.tile([128, 2], F32, name="bk_sb", tag="bk_sb")
        bv_sb = cpool.tile([1, GV], F32, name="bv_sb", tag="bv_sb")
        kpm_sb = cpool.tile([128, NT], F32, name="kpm_sb", tag="kpm_sb")

        # ---- long-lived activations ----
        qT = [keep.tile([128, T], ATT, name=f"qT{m}", tag=f"qT{m}") for m in range(2)]
        kT = [keep.tile([128, T], ATT, name=f"kT{m}", tag=f"kT{m}") for m in range(2)]
        vt = [keep.tile([128, GV], ATT, name=f"vt{t}", tag=f"vt{t}") for t in range(NT)]
        attT = [keep.tile([128, T], ATT, name=f"attT{m}", tag=f"attT{m}") for m in range(2)]
        wo_r = [keep.tile([128, D], ATT, name=f"wo{i}", tag=f"wo{i}") for i in range(2)]

        # input H^T, token-sliced into three merged tiles (k-chunk-major
        # within each token range) so projections can start at ~25% of the
        # HT fill while the DMA count stays small.
        ht_q0m = work.tile([128, 4096], BF16, name="ht_q0m", tag="ht_q0m")
        ht_q1m = work.tile([128, 4096], BF16, name="ht_q1m", tag="ht_q1m")
        ht_h1m = work.tile([128, 8192], BF16, name="ht_h1m", tag="ht_h1m")

        def ht_blk(k, n):
            # [128, 512] slice of H^T k-chunk covering tokens n*512:(n+1)*512
            if n == 0:
                return ht_q0m[:, k * 512:(k + 1) * 512]
            if n == 1:
                return ht_q1m[:, k * 512:(k + 1) * 512]
            return ht_h1m[:, k * 1024 + (n - 2) * 512:k * 1024 + (n - 1) * 512]

        def ht_tok(k, t):
            # [128, 128] slice covering token chunk t
            n, o = divmod(t, 4)
            return ht_blk(k, n)[:, o * 128:(o + 1) * 128]

        wq_r = work.tile([128, NKD * GD], BF16, name="wq_r", tag="wq_r")
        wk_r = work.tile([128, NKD * GD], BF16, name="wk_r", tag="wk_r")
        wv_r = work.tile([128, NKD * GV], BF16, name="wv_r", tag="wv_r")

        # ---- input DMA, spread across queues ----
        # vector queue: warmup memset; gpsimd: tiny per-core constants
        warm = cpool.tile([128, 512], BF16, name="warm", tag="warm")
        nc.vector.memset(warm[:], 0.0)
        nc.gpsimd.dma_start(bv_sb[:], bvS[:])
        nc.gpsimd.dma_start(bq_sb[:], bqc[:])
        nc.gpsimd.dma_start(bk_sb[:], bkc[:])
        nc.gpsimd.dma_start(kpm_sb[:], kpm[:])
        # ALL input transfers go on ONE queue in strict need order: with two
        # queues the rings race into not-yet-needed bytes and starve the
        # bootstrap-critical set (everything shares the ~358 GB/s HBM pipe).
        # Need order: m=0 weight halves + wv + HT first quarter (~2 MB,
        # feeds the bootstrap), HT second quarter, HT back half, m=1 weight
        # halves, Wo.
        HW = NKD * 128  # 1024: one m-half of wq/wk

        def ht_dma(dst, rows, cols, nk):
            nc.scalar.dma_start(
                dst.rearrange("p (k t) -> p k t", k=nk),
                HT[rows[0]:rows[1], cols[0]:cols[1]]
                .rearrange("(k p) t -> p k t", k=nk),
            )

        nc.scalar.dma_start(wq_r[:, 0:HW], WqR[:, 0:HW])
        ht_dma(ht_q0m[:, 0:2048], (0, 512), (0, 512), 4)
        ht_dma(ht_q0m[:, 2048:4096], (512, 1024), (0, 512), 4)
        nc.scalar.dma_start(wk_r[:, 0:HW], WkR[:, 0:HW])
        nc.scalar.dma_start(wv_r[:], WvR[:])
        ht_dma(ht_q1m[:, :], (0, 1024), (512, 1024), 8)
        ht_dma(ht_h1m[:, 0:4096], (0, 512), (1024, 2048), 4)
        ht_dma(ht_h1m[:, 4096:8192], (512, 1024), (1024, 2048), 4)
        nc.scalar.dma_start(wq_r[:, HW:2 * HW], WqR[:, HW:2 * HW])
        nc.scalar.dma_start(wk_r[:, HW:2 * HW], WkR[:, HW:2 * HW])
        for i in range(2):
            nc.scalar.dma_start(wo_r[i][:], WoS[i * 128:(i + 1) * 128, :])
        # gpsimd queue: masks/constants, bv broadcast (stays off the HBM
        # critical path; later it only issues output DMAs)
        tri = cpool.tile([128, 128], ATT, name="tri", tag="tri")
        nc.gpsimd.memset(tri[:], 1.0)
        nc.gpsimd.affine_select(
            out=tri[:], in_=tri[:], compare_op=OP.is_ge, fill=0.0,
            base=0, pattern=[[1, 128]], channel_multiplier=-1,
        )
        # bv broadcast across partitions: [128, GV]
        bvrow = cpool.tile([128, GV], F32, name="bvrow", tag="bvrow")
        nc.gpsimd.partition_broadcast(bvrow[:], bv_sb[:])

        # HAM warmup: keep the PE array busy during the input-DMA fill so
        # the clock gate reaches K=8/8 before real matmuls start (zero
        # data, the results are never read).
        for _ in range(N_WARM):
            wp = ps_sm.tile([128, 512], F32, name="wp", tag="smps", bufs=2)
            nc.tensor.matmul(wp[:], warm[:, 0:128], warm[:], start=True, stop=True)

        # ---- filler work units (PE work injected between attention chunks) ----
        def proj_qk(w_r, dest, bias_sb, m, n, c0=0, c1=512):
            # dest[m][:, n*512+c0 : n*512+c1] = sum_k W_k[:, m-block].T @ ht_k + bias
            w = c1 - c0
            ps = ps_sm.tile([128, 512], F32, name="pp", tag="smps", bufs=2)
            for k in range(NKD):
                nc.tensor.matmul(
                    ps[:, 0:w],
                    w_r[:, (m * NKD + k) * 128:(m * NKD + k) * 128 + 128],
                    ht_blk(k, n)[:, c0:c1],
                    start=(k == 0), stop=(k == NKD - 1),
                )
            # bias add folded into the PSUM->SBUF copy (DVE, per-partition scalar)
            nc.vector.tensor_scalar_add(
                dest[m][:, n * 512 + c0:n * 512 + c1], ps[:, 0:w],
                bias_sb[:, m:m + 1]
            )

        def proj_v(t):
            # vt[t] per-head blocks = (sum_k ht_k_t.T @ WvS_k + bv) * kpm
            vp = ps_sm.tile([128, 512], F32, name="vp", tag="smps", bufs=2)
            for k in range(NKD):
                nc.tensor.matmul(
                    vp[:, 0:GV],
                    ht_tok(k, t),
                    wv_r[:, k * GV:(k + 1) * GV],
                    start=(k == 0), stop=(k == NKD - 1),
                )
            nc.vector.tensor_tensor(
                vt[t][:], vp[:, 0:GV], bvrow[:], op=OP.add,
            )
            nc.vector.tensor_scalar_mul(vt[t][:], vt[t][:], kpm_sb[:, t:t + 1])

        def out_half(t, n):
            # O[t-chunk, n-half] = sum_hp attT[hp][:, t-chunk].T @ WoS[hp][:, n-half]
            ot = work.tile([128, 512], BF16, name="ot", tag="ot", bufs=4)
            op = ps_sm.tile([128, 512], F32, name="op", tag="smps", bufs=2)
            for hp in range(2):
                nc.tensor.matmul(
                    op[:],
                    attT[hp][:, t * 128:(t + 1) * 128],
                    wo_r[hp][:, n * 512:(n + 1) * 512],
                    start=(hp == 0), stop=(hp == 1),
                )
            # PSUM->SBUF copy on the scalar engine (Identity shares the Exp
            # ACT table): frees the DVE, which runs the normalize multiplies.
            # Output DMA issues from the gpsimd queue (idle mid-kernel).
            nc.scalar.activation(ot[:], op[:], AF.Identity)
            nc.gpsimd.dma_start(O[t * 128:(t + 1) * 128, n * 512:(n + 1) * 512],
                                ot[:])

        fillers = []

        def pop_filler():
            if fillers:
                fillers.pop(0)()

        # ---- attention block machinery ----
        def normalize(hp, J, at):
            # zau: unnormalized A^T V rows (0:64) + denominator row (64)
            zaus = []
            for hh in range(2):
                zau = work.tile([65, 512], F32, name="zau", tag="zau", bufs=4)
                nc.vector.tensor_copy(zau[:], at[hh][0:65, :])
                zaus.append(zau)
            bi = hp * 4 + J
            # exact reciprocal on a partition-packed [128, 8] tile: bounce the
            # two denominator rows through DRAM (engines cannot cross
            # partitions; DMA can). 8 elem/lane keeps the iterative divide
            # at ~130 ns instead of 4.3 us on a [1, 512] row.
            for hh in range(2):
                nc.sync.dma_start(zd[bi:bi + 1, hh * 512:(hh + 1) * 512],
                                    zaus[hh][64:65, :])
            zp = work.tile([128, 8], F32, name="zp", tag="zp", bufs=2)
            nc.sync.dma_start(
                zp[:], zd[bi:bi + 1, :].rearrange("p (a b) -> (p a) b", b=8)
            )
            rp = work.tile([128, 8], F32, name="rp", tag="rp", bufs=2)
            nc.vector.reciprocal(rp[:], zp[:])
            nc.sync.dma_start(
                rd[bi:bi + 1, :].rearrange("p (a b) -> (p a) b", b=8), rp[:]
            )
            for hh in range(2):
                zau = zaus[hh]
                # partition-broadcast 1/z straight out of DRAM with a
                # 0-stride DMA read (frees gpsimd, one fewer serial hop)
                rb = work.tile([64, 512], F32, name="rb", tag="rb", bufs=4)
                nc.sync.dma_start(
                    rb[:],
                    rd[bi:bi + 1, hh * 512:(hh + 1) * 512]
                    .squeeze(0).partition_broadcast(64),
                )
                nc.vector.tensor_tensor(
                    attT[hp][hh * 64:(hh + 1) * 64, J * 512:(J + 1) * 512],
                    zau[0:64, :],
                    rb[:],
                    op=OP.mult,
                )

        def block(J, hp):
            at = [
                ps_at.tile([128, 512], F32, name=f"at{hh}", tag="av", bufs=2)
                for hh in range(2)
            ]
            # diagonal chunk first (full width, opens PSUM accumulation),
            # then off-diagonals, then narrow diagonals.
            kcs = [4 * J] + list(range(4 * J)) + [4 * J + i for i in range(1, 4)]

            def issue_sc_exp(kc):
                off = max(0, 128 * (kc - 4 * J))
                w = 512 - off
                sc = ps_sc.tile([128, 1024], F32, name="sc", tag="sc", bufs=2)
                for hh in range(2):
                    nc.tensor.matmul(
                        sc[:, hh * 512:hh * 512 + w],
                        kT[hp][hh * 64:(hh + 1) * 64, kc * 128:(kc + 1) * 128],
                        qT[hp][hh * 64:(hh + 1) * 64, J * 512 + off:(J + 1) * 512],
                        start=True, stop=True,
                        tile_position=(hh * 64, 0),
                    )
                ex = work.tile([128, 1024], ATT, name="ex", tag="ex", bufs=8)
                nc.scalar.activation(
                    ex[:].rearrange("p (h c) -> p h c", c=512)[:, :, 0:w],
                    sc[:].rearrange("p (h c) -> p h c", c=512)[:, :, 0:w],
                    AF.Exp, scale=0.125,
                )
                if off or kc == 4 * J:
                    for hh in range(2):
                        nc.vector.tensor_tensor(
                            ex[:, hh * 512:hh * 512 + 128],
                            ex[:, hh * 512:hh * 512 + 128],
                            tri[:],
                            op=OP.mult,
                        )
                return ex

            def issue_av(kc, ex, first, last):
                off = max(0, 128 * (kc - 4 * J))
                w = 512 - off
                for hh in range(2):
                    h = 2 * hp + hh
                    nc.tensor.matmul(
                        at[hh][0:65, off:512],
                        vt[kc][:, h * 65:(h + 1) * 65],
                        ex[:, hh * 512:hh * 512 + w],
                        start=first, stop=last,
                    )

            prev = None
            for ti, kc in enumerate(kcs):
                ex = issue_sc_exp(kc)
                pop_filler()
                if prev is not None:
                    issue_av(prev[0], prev[1], first=(prev[2] == 0), last=False)
                prev = (kc, ex, ti)
            issue_av(prev[0], prev[1], first=(prev[2] == 0), last=True)
            normalize(hp, J, at)

        # ---- bootstrap projections for the first diagonal chunk ----
        # minimal set for the first score/AV chunk: full q block, the first
        # 128 key columns, vt[0]; everything else becomes fillers so the PE
        # never idles (idling drops the HAM clock gate to K=4).
        proj_qk(wq_r, qT, bq_sb, 0, 0)
        proj_qk(wk_r, kT, bk_sb, 0, 0, 0, 128)
        proj_v(0)

        def qk(hp_, n_):
            fillers.append(lambda: proj_qk(wq_r, qT, bq_sb, hp_, n_))
            fillers.append(lambda: proj_qk(wk_r, kT, bk_sb, hp_, n_))

        # filler schedule, matched to each block's pop budget (block(J,hp)
        # pops 4J+4 fillers; two extra boundary pops follow block(0,0)) and
        # to the token-sliced HT DMA arrival order. Each proj must be popped
        # no later than its first consumer chunk in the block pipeline.
        fillers.append(lambda: proj_qk(wk_r, kT, bk_sb, 0, 0, 128, 512))
        for t in (1, 2, 3):
            fillers.append(lambda t=t: proj_v(t))
        qk(0, 1)                                     # boundary pops
        for t in (4, 5, 6, 7):
            fillers.append(lambda t=t: proj_v(t))
        qk(0, 2)
        qk(0, 3)
        for t in (8, 9, 10, 11):
            fillers.append(lambda t=t: proj_v(t))
        for n in reversed(range(4)):
            qk(1, n)
        for t in (12, 13, 14, 15):
            fillers.append(lambda t=t: proj_v(t))

        # ---- main pass ----
        # hp1 runs J descending: the big J=3 block comes first (making its
        # output-projection fillers available early) and the small J=0 block
        # lands last, shortening the final normalize->out tail.
        for hp, Js in ((0, range(NJ)), (1, reversed(range(NJ)))):
            for J in Js:
                block(J, hp)
                if hp == 0 and J == 0:
                    pop_filler()
                    pop_filler()
                if hp == 1:
                    # attT for both head pairs at J is now final
                    for t in range(4 * J, 4 * J + 4):
                        for n in range(2):
                            fillers.append(lambda t=t, n=n: out_half(t, n))
        while fillers:
            fillers.pop(0)()

    nc.compile()
    _NC_CACHE["nc"] = nc
    return nc


def _prep_core_inputs(H, key_padding_mask, Wq, bq, Wk, bk, Wv, bv, Wo, bo):
    keep = 1.0 - np.asarray(key_padding_mask, dtype=np.float32)  # [B, T]
    bf = ml_dtypes.bfloat16
    in_maps = []
    def to_sbuf_layout(WT):
        # [D, G] -> [128, NKD*G]: row p holds k-chunk-major slices
        G = WT.shape[1]
        return np.ascontiguousarray(
            WT.reshape(NKD, 128, G).transpose(1, 0, 2).reshape(128, NKD * G))

    def to_sbuf_layout_m(WT):
        # [D, 256] -> [128, 2*NKD*128]: m-half-major, then k-chunk-major
        return np.ascontiguousarray(
            WT.reshape(NKD, 128, 2, 128).transpose(1, 2, 0, 3)
            .reshape(128, 2 * NKD * 128))

    for c in range(8):
        b, g = divmod(c, 4)
        sl = slice(g * GD, (g + 1) * GD)
        WvT = Wv[sl].T  # [D, GD]
        WvS = np.zeros((D, GV), dtype=np.float32)
        bvS = np.zeros((1, GV), dtype=np.float32)
        for h in range(HPC):
            WvS[:, h * 65:h * 65 + 64] = WvT[:, h * 64:(h + 1) * 64]
            bvS[0, h * 65:h * 65 + 64] = bv[sl][h * 64:(h + 1) * 64]
            bvS[0, h * 65 + 64] = 1.0
        in_maps.append({
            "HT": np.ascontiguousarray(H[b].T).astype(bf),
            "WqR": to_sbuf_layout_m(Wq[sl].T).astype(bf),
            "WkR": to_sbuf_layout_m(Wk[sl].T).astype(bf),
            "WvR": to_sbuf_layout(WvS).astype(bf),
            "WoS": np.ascontiguousarray(Wo[:, sl].T).astype(bf),
            "bqc": np.ascontiguousarray(bq[sl].reshape(2, 128).T.astype(np.float32)),
            "bkc": np.ascontiguousarray(bk[sl].reshape(2, 128).T.astype(np.float32)),
            "bvS": bvS,
            "kpm": np.ascontiguousarray(keep[b].reshape(NT, 128).T),
        })
    return in_maps


def kernel(H, key_padding_mask, Wq, bq, Wk, bk, Wv, bv, Wo, bo, _run_kwargs=None):
    H = np.asarray(H, dtype=np.float32)
    Wq = np.asarray(Wq, dtype=np.float32)
    Wk = np.asarray(Wk, dtype=np.float32)
    Wv = np.asarray(Wv, dtype=np.float32)
    Wo = np.asarray(Wo, dtype=np.float32)
    bq = np.asarray(bq, dtype=np.float32)
    bk = np.asarray(bk, dtype=np.float32)
    bv = np.asarray(bv, dtype=np.float32)
    bo = np.asarray(bo, dtype=np.float32)

    nc = build()
    in_maps = _prep_core_inputs(H, key_padding_mask, Wq, bq, Wk, bk, Wv, bv, Wo, bo)
    res = bass_utils.run_bass_kernel_spmd(
        nc, in_maps, core_ids=list(range(8)), **(_run_kwargs or {})
    )
    out = np.zeros((B, T, D), dtype=np.float32)
    for c in range(8):
        out[c // 4] += res.results[c]["O"].astype(np.float32)
    out += bo
    if _run_kwargs:
        kernel.last_result = res
    return out


# revision 39
# speedup vs baseline: 1.0173x; 1.0104x over previous
"""Multi-head self-attention (B=2, T=2048, D=1024, 16 heads) on 8 TRN2 cores.

Sharding: core c = (b, g) with b = c // 4 (batch), g = c % 4 (head group of 4).
Each core computes q/k/v projections for its 4 heads, causal softmax
attention, and a partial output projection (its 256 columns of the
concat-head dim against Wo). Host sums the 4 partials per batch and adds bo.

v3: same fully-interleaved pass as v2 (attention chunk pipeline with
projection / output-projection fillers), plus:
  - token-sliced HT input DMA across 3 queues so the bootstrap
    projections start at ~25% of the HT fill instead of 100%;
  - compact vt tiles [128, 260] (4 heads x 65: 64 v dims + a ones
    column that makes AV emit softmax denominators) -- no zero padding,
    no gpsimd memsets;
  - the final block's softmax normalization uses an ACT-table
    reciprocal on the PSUM denominator row + a PE ones-matmul
    partition-broadcast instead of the DRAM-bounce + gpsimd path,
    cutting the serial tail;
  - per-half output tiles so the two DMA halves of each O row chunk
    are independent.

Per-core layout:
  qT/kT [128, 2048] bf16: rows = 2 heads x 64 dims, cols = tokens.
  vt[t] [128, 260] bf16: rows = 128 key tokens of chunk t, cols =
     4 heads x 65 (64 v dims + a 1.0 column).
  attT [128, 2048] bf16 per head pair: normalized A^T V rows.
  O [2048, 1024] bf16 partial output, summed on host in f32.
"""

import ml_dtypes
import numpy as np

import concourse.bass as bass
import concourse.tile as tile
from concourse import bacc, mybir
from concourse import bass_utils
from contextlib import ExitStack

F32 = mybir.dt.float32
BF16 = mybir.dt.bfloat16
ATT = BF16
AF = mybir.ActivationFunctionType
OP = mybir.AluOpType

B, T, D = 2, 2048, 1024
NH, DH = 16, 64
HPC = 4            # heads per core
GD = HPC * DH      # 256, group dim
GV = HPC * (DH + 1)  # 260, v tile width (compact, 65 per head)
NKD = D // 128     # 8 K-chunks for projections
NT = T // 128      # 16 token chunks
NJ = T // 512      # 4 query blocks
N_WARM = 12        # HAM clock-ramp warmup matmuls

_NC_CACHE = {}


def build():
    if "nc" in _NC_CACHE:
        return _NC_CACHE["nc"]
    nc = bacc.Bacc("TRN2", target_bir_lowering=False, debug=False, num_devices=8)

    HT = nc.dram_tensor("HT", [D, T], BF16, kind="ExternalInput").ap()
    # weights pre-permuted on the host to the SBUF layout (m-major, then
    # k-chunk-major) so input DMAs are plain streams, not slow gathers --
    # and the m=1 half can be deferred past the bootstrap-critical bytes
    WqR = nc.dram_tensor("WqR", [128, NKD * GD], BF16, kind="ExternalInput").ap()
    WkR = nc.dram_tensor("WkR", [128, NKD * GD], BF16, kind="ExternalInput").ap()
    WvR = nc.dram_tensor("WvR", [128, NKD * GV], BF16, kind="ExternalInput").ap()
    WoS = nc.dram_tensor("WoS", [GD, D], BF16, kind="ExternalInput").ap()
    bqc = nc.dram_tensor("bqc", [128, 2], F32, kind="ExternalInput").ap()
    bkc = nc.dram_tensor("bkc", [128, 2], F32, kind="ExternalInput").ap()
    bvS = nc.dram_tensor("bvS", [1, GV], F32, kind="ExternalInput").ap()
    kpm = nc.dram_tensor("kpm", [128, NT], F32, kind="ExternalInput").ap()
    O = nc.dram_tensor("O", [T, D], BF16, kind="ExternalOutput").ap()
    zd = nc.dram_tensor("zd", [8, 1024], F32, kind="Internal").ap()
    rd = nc.dram_tensor("rd", [8, 1024], F32, kind="Internal").ap()

    with tile.TileContext(nc) as tc, ExitStack() as octx:
        cpool = octx.enter_context(tc.tile_pool(name="const", bufs=1))
        keep = octx.enter_context(tc.tile_pool(name="keep", bufs=1))
        work = octx.enter_context(tc.tile_pool(name="work", bufs=1))
        ps_sc = octx.enter_context(tc.tile_pool(name="ps_sc", bufs=1, space="PSUM"))
        ps_at = octx.enter_context(tc.tile_pool(name="ps_at", bufs=1, space="PSUM"))
        ps_sm = octx.enter_context(tc.tile_pool(name="ps_sm", bufs=1, space="PSUM"))

        # ---- constants ----
        bq_sb = cpool.tile([128, 2], F32, name="bq_sb", tag="bq_sb")
        bk_sb = cpool.tile([128, 2], F32, name="bk_sb", tag="bk_sb")
        bv_sb = cpool.tile([1, GV], F32, name="bv_sb", tag="bv_sb")
        kpm_sb = cpool.tile([128, NT], F32, name="kpm_sb", tag="kpm_sb")

        # ---- long-lived activations ----
        qT = [keep.tile([128, T], ATT, name=f"qT{m}", tag=f"qT{m}") for m in range(2)]
        kT = [keep.tile([128, T], ATT, name=f"kT{m}", tag=f"kT{m}") for m in range(2)]
        vt = [keep.tile([128, GV], ATT, name=f"vt{t}", tag=f"vt{t}") for t in range(NT)]
        attT = [keep.tile([128, T], ATT, name=f"attT{m}", tag=f"attT{m}") for m in range(2)]
        wo_r = [keep.tile([128, D], ATT, name=f"wo{i}", tag=f"wo{i}") for i in range(2)]

        # input H^T, token-sliced into four merged tiles (one per 512-token
        # quarter, k-chunk-major inside) so projections can start at ~25%
        # of the HT fill and each filler only depends on one quarter.
        ht_q = [work.tile([128, 4096], BF16, name=f"ht_q{n}", tag=f"ht_q{n}")
                for n in range(4)]

        def ht_blk(k, n):
            # [128, 512] slice of H^T k-chunk covering tokens n*512:(n+1)*512
            return ht_q[n][:, k * 512:(k + 1) * 512]

        def ht_tok(k, t):
            # [128, 128] slice covering token chunk t
            n, o = divmod(t, 4)
            return ht_blk(k, n)[:, o * 128:(o + 1) * 128]

        wq_r = work.tile([128, NKD * GD], BF16, name="wq_r", tag="wq_r")
        wk_r = work.tile([128, NKD * GD], BF16, name="wk_r", tag="wk_r")
        wv_r = work.tile([128, NKD * GV], BF16, name="wv_r", tag="wv_r")

        # ---- input DMA, spread across queues ----
        # vector queue: warmup memset; gpsimd: tiny per-core constants
        warm = cpool.tile([128, 512], BF16, name="warm", tag="warm")
        nc.vector.memset(warm[:], 0.0)
        nc.gpsimd.dma_start(bv_sb[:], bvS[:])
        nc.gpsimd.dma_start(bq_sb[:], bqc[:])
        nc.gpsimd.dma_start(bk_sb[:], bkc[:])
        nc.gpsimd.dma_start(kpm_sb[:], kpm[:])
        # ALL input transfers go on ONE queue in strict need order: with two
        # queues the rings race into not-yet-needed bytes and starve the
        # bootstrap-critical set (everything shares the ~358 GB/s HBM pipe).
        # Need order: m=0 weight halves + wv + HT first quarter (~2 MB,
        # feeds the bootstrap), HT second quarter, HT back half, m=1 weight
        # halves, Wo.
        HW = NKD * 128  # 1024: one m-half of wq/wk

        def ht_dma(dst, rows, cols, nk):
            nc.scalar.dma_start(
                dst.rearrange("p (k t) -> p k t", k=nk),
                HT[rows[0]:rows[1], cols[0]:cols[1]]
                .rearrange("(k p) t -> p k t", k=nk),
            )

        nc.scalar.dma_start(wq_r[:, 0:HW], WqR[:, 0:HW])
        ht_dma(ht_q[0][:, 0:2048], (0, 512), (0, 512), 4)
        ht_dma(ht_q[0][:, 2048:4096], (512, 1024), (0, 512), 4)
        nc.scalar.dma_start(wk_r[:, 0:HW], WkR[:, 0:HW])
        nc.scalar.dma_start(wv_r[:], WvR[:])
        ht_dma(ht_q[1][:, :], (0, 1024), (512, 1024), 8)
        ht_dma(ht_q[2][:, 0:2048], (0, 512), (1024, 1536), 4)
        ht_dma(ht_q[2][:, 2048:4096], (512, 1024), (1024, 1536), 4)
        ht_dma(ht_q[3][:, 0:2048], (0, 512), (1536, 2048), 4)
        ht_dma(ht_q[3][:, 2048:4096], (512, 1024), (1536, 2048), 4)
        nc.scalar.dma_start(wq_r[:, HW:2 * HW], WqR[:, HW:2 * HW])
        nc.scalar.dma_start(wk_r[:, HW:2 * HW], WkR[:, HW:2 * HW])
        for i in range(2):
            nc.scalar.dma_start(wo_r[i][:], WoS[i * 128:(i + 1) * 128, :])
        # gpsimd queue: masks/constants, bv broadcast (stays off the HBM
        # critical path; later it only issues output DMAs)
        tri = cpool.tile([128, 128], ATT, name="tri", tag="tri")
        nc.gpsimd.memset(tri[:], 1.0)
        nc.gpsimd.affine_select(
            out=tri[:], in_=tri[:], compare_op=OP.is_ge, fill=0.0,
            base=0, pattern=[[1, 128]], channel_multiplier=-1,
        )
        # bv broadcast across partitions: [128, GV]
        bvrow = cpool.tile([128, GV], F32, name="bvrow", tag="bvrow")
        nc.gpsimd.partition_broadcast(bvrow[:], bv_sb[:])

        # HAM warmup: keep the PE array busy during the input-DMA fill so
        # the clock gate reaches K=8/8 before real matmuls start (zero
        # data, the results are never read).
        for _ in range(N_WARM):
            wp = ps_sm.tile([128, 512], F32, name="wp", tag="smps", bufs=2)
            nc.tensor.matmul(wp[:], warm[:, 0:128], warm[:], start=True, stop=True)

        # ---- filler work units (PE work injected between attention chunks) ----
        def proj_qk(w_r, dest, bias_sb, m, n, c0=0, c1=512):
            # dest[m][:, n*512+c0 : n*512+c1] = sum_k W_k[:, m-block].T @ ht_k + bias
            w = c1 - c0
            ps = ps_sm.tile([128, 512], F32, name="pp", tag="smps", bufs=2)
            for k in range(NKD):
                nc.tensor.matmul(
                    ps[:, 0:w],
                    w_r[:, (m * NKD + k) * 128:(m * NKD + k) * 128 + 128],
                    ht_blk(k, n)[:, c0:c1],
                    start=(k == 0), stop=(k == NKD - 1),
                )
            # bias add folded into the PSUM->SBUF copy (DVE, per-partition scalar)
            nc.vector.tensor_scalar_add(
                dest[m][:, n * 512 + c0:n * 512 + c1], ps[:, 0:w],
                bias_sb[:, m:m + 1]
            )

        def proj_v(t):
            # vt[t] per-head blocks = (sum_k ht_k_t.T @ WvS_k + bv) * kpm
            vp = ps_sm.tile([128, 512], F32, name="vp", tag="smps", bufs=2)
            for k in range(NKD):
                nc.tensor.matmul(
                    vp[:, 0:GV],
                    ht_tok(k, t),
                    wv_r[:, k * GV:(k + 1) * GV],
                    start=(k == 0), stop=(k == NKD - 1),
                )
            nc.vector.tensor_tensor(
                vt[t][:], vp[:, 0:GV], bvrow[:], op=OP.add,
            )
            nc.vector.tensor_scalar_mul(vt[t][:], vt[t][:], kpm_sb[:, t:t + 1])

        def out_half(t, n):
            # O[t-chunk, n-half] = sum_hp attT[hp][:, t-chunk].T @ WoS[hp][:, n-half]
            ot = work.tile([128, 512], BF16, name="ot", tag="ot", bufs=4)
            op = ps_sm.tile([128, 512], F32, name="op", tag="smps", bufs=2)
            for hp in range(2):
                nc.tensor.matmul(
                    op[:],
                    attT[hp][:, t * 128:(t + 1) * 128],
                    wo_r[hp][:, n * 512:(n + 1) * 512],
                    start=(hp == 0), stop=(hp == 1),
                )
            # PSUM->SBUF copy on the scalar engine (Identity shares the Exp
            # ACT table): frees the DVE, which runs the normalize multiplies.
            # Output DMA issues from the gpsimd queue (idle mid-kernel).
            nc.scalar.activation(ot[:], op[:], AF.Identity)
            nc.gpsimd.dma_start(O[t * 128:(t + 1) * 128, n * 512:(n + 1) * 512],
                                ot[:])

        fillers = []

        def pop_filler():
            if fillers:
                fillers.pop(0)()

        # ---- attention block machinery ----
        def normalize(hp, J, at):
            # zau: unnormalized A^T V rows (0:64) + denominator row (64)
            zaus = []
            for hh in range(2):
                zau = work.tile([65, 512], F32, name="zau", tag="zau", bufs=4)
                nc.vector.tensor_copy(zau[:], at[hh][0:65, :])
                zaus.append(zau)
            bi = hp * 4 + J
            # exact reciprocal on a partition-packed [128, 8] tile: bounce the
            # two denominator rows through DRAM (engines cannot cross
            # partitions; DMA can). 8 elem/lane keeps the iterative divide
            # at ~130 ns instead of 4.3 us on a [1, 512] row.
            for hh in range(2):
                nc.sync.dma_start(zd[bi:bi + 1, hh * 512:(hh + 1) * 512],
                                    zaus[hh][64:65, :])
            zp = work.tile([128, 8], F32, name="zp", tag="zp", bufs=2)
            nc.sync.dma_start(
                zp[:], zd[bi:bi + 1, :].rearrange("p (a b) -> (p a) b", b=8)
            )
            rp = work.tile([128, 8], F32, name="rp", tag="rp", bufs=2)
            nc.vector.reciprocal(rp[:], zp[:])
            nc.sync.dma_start(
                rd[bi:bi + 1, :].rearrange("p (a b) -> (p a) b", b=8), rp[:]
            )
            for hh in range(2):
                zau = zaus[hh]
                # partition-broadcast 1/z straight out of DRAM with a
                # 0-stride DMA read (frees gpsimd, one fewer serial hop)
                rb = work.tile([64, 512], F32, name="rb", tag="rb", bufs=4)
                nc.sync.dma_start(
                    rb[:],
                    rd[bi:bi + 1, hh * 512:(hh + 1) * 512]
                    .squeeze(0).partition_broadcast(64),
                )
                nc.vector.tensor_tensor(
                    attT[hp][hh * 64:(hh + 1) * 64, J * 512:(J + 1) * 512],
                    zau[0:64, :],
                    rb[:],
                    op=OP.mult,
                )

        def block(J, hp):
            at = [
                ps_at.tile([128, 512], F32, name=f"at{hh}", tag="av", bufs=2)
                for hh in range(2)
            ]
            # diagonal chunk first (full width, opens PSUM accumulation),
            # then off-diagonals, then narrow diagonals.
            kcs = [4 * J] + list(range(4 * J)) + [4 * J + i for i in range(1, 4)]

            def issue_sc_exp(kc):
                off = max(0, 128 * (kc - 4 * J))
                w = 512 - off
                sc = ps_sc.tile([128, 1024], F32, name="sc", tag="sc", bufs=2)
                for hh in range(2):
                    nc.tensor.matmul(
                        sc[:, hh * 512:hh * 512 + w],
                        kT[hp][hh * 64:(hh + 1) * 64, kc * 128:(kc + 1) * 128],
                        qT[hp][hh * 64:(hh + 1) * 64, J * 512 + off:(J + 1) * 512],
                        start=True, stop=True,
                        tile_position=(hh * 64, 0),
                    )
                ex = work.tile([128, 1024], ATT, name="ex", tag="ex", bufs=8)
                nc.scalar.activation(
                    ex[:].rearrange("p (h c) -> p h c", c=512)[:, :, 0:w],
                    sc[:].rearrange("p (h c) -> p h c", c=512)[:, :, 0:w],
                    AF.Exp, scale=0.125,
                )
                if off or kc == 4 * J:
                    for hh in range(2):
                        nc.vector.tensor_tensor(
                            ex[:, hh * 512:hh * 512 + 128],
                            ex[:, hh * 512:hh * 512 + 128],
                            tri[:],
                            op=OP.mult,
                        )
                return ex

            def issue_av(kc, ex, first, last):
                off = max(0, 128 * (kc - 4 * J))
                w = 512 - off
                for hh in range(2):
                    h = 2 * hp + hh
                    nc.tensor.matmul(
                        at[hh][0:65, off:512],
                        vt[kc][:, h * 65:(h + 1) * 65],
                        ex[:, hh * 512:hh * 512 + w],
                        start=first, stop=last,
                    )

            prev = None
            for ti, kc in enumerate(kcs):
                ex = issue_sc_exp(kc)
                pop_filler()
                if prev is not None:
                    issue_av(prev[0], prev[1], first=(prev[2] == 0), last=False)
                prev = (kc, ex, ti)
            issue_av(prev[0], prev[1], first=(prev[2] == 0), last=True)
            normalize(hp, J, at)

        # ---- bootstrap projections for the first diagonal chunk ----
        # minimal set for the first score/AV chunk: full q block, the first
        # 128 key columns, vt[0]; everything else becomes fillers so the PE
        # never idles (idling drops the HAM clock gate to K=4).
        proj_qk(wq_r, qT, bq_sb, 0, 0)
        proj_qk(wk_r, kT, bk_sb, 0, 0, 0, 128)
        proj_v(0)

        def qk(hp_, n_):
            fillers.append(lambda: proj_qk(wq_r, qT, bq_sb, hp_, n_))
            fillers.append(lambda: proj_qk(wk_r, kT, bk_sb, hp_, n_))

        # filler schedule, matched to each block's pop budget (block(J,hp)
        # pops 4J+4 fillers; two extra boundary pops follow block(0,0)) and
        # to the token-sliced HT DMA arrival order. Each proj must be popped
        # no later than its first consumer chunk in the block pipeline.
        fillers.append(lambda: proj_qk(wk_r, kT, bk_sb, 0, 0, 128, 512))
        for t in (1, 2, 3):
            fillers.append(lambda t=t: proj_v(t))
        qk(0, 1)                                     # boundary pops
        for t in (4, 5, 6, 7):                       # block(1,0) pops
            fillers.append(lambda t=t: proj_v(t))
        qk(0, 2)
        for t in (8, 9):
            fillers.append(lambda t=t: proj_v(t))
        for t in (10, 11):                           # block(2,0) pops
            fillers.append(lambda t=t: proj_v(t))
        qk(0, 3)
        for n in reversed(range(4)):
            qk(1, n)
        for t in (12, 13, 14, 15):                   # block(3,0) pops
            fillers.append(lambda t=t: proj_v(t))

        # ---- main pass ----
        # hp1 runs J descending: the big J=3 block comes first (making its
        # output-projection fillers available early) and the small J=0 block
        # lands last, shortening the final normalize->out tail.
        for hp, Js in ((0, range(NJ)), (1, reversed(range(NJ)))):
            for J in Js:
                block(J, hp)
                if hp == 0 and J == 0:
                    pop_filler()
                    pop_filler()
                if hp == 1:
                    # attT for both head pairs at J is now final
                    for t in range(4 * J, 4 * J + 4):
                        for n in range(2):
                            fillers.append(lambda t=t, n=n: out_half(t, n))
        while fillers:
            fillers.pop(0)()

    nc.compile()
    _NC_CACHE["nc"] = nc
    return nc


def _prep_core_inputs(H, key_padding_mask, Wq, bq, Wk, bk, Wv, bv, Wo, bo):
    keep = 1.0 - np.asarray(key_padding_mask, dtype=np.float32)  # [B, T]
    bf = ml_dtypes.bfloat16
    in_maps = []
    def to_sbuf_layout(WT):
        # [D, G] -> [128, NKD*G]: row p holds k-chunk-major slices
        G = WT.shape[1]
        return np.ascontiguousarray(
            WT.reshape(NKD, 128, G).transpose(1, 0, 2).reshape(128, NKD * G))

    def to_sbuf_layout_m(WT):
        # [D, 256] -> [128, 2*NKD*128]: m-half-major, then k-chunk-major
        return np.ascontiguousarray(
            WT.reshape(NKD, 128, 2, 128).transpose(1, 2, 0, 3)
            .reshape(128, 2 * NKD * 128))

    for c in range(8):
        b, g = divmod(c, 4)
        sl = slice(g * GD, (g + 1) * GD)
        WvT = Wv[sl].T  # [D, GD]
        WvS = np.zeros((D, GV), dtype=np.float32)
        bvS = np.zeros((1, GV), dtype=np.float32)
        for h in range(HPC):
            WvS[:, h * 65:h * 65 + 64] = WvT[:, h * 64:(h + 1) * 64]
            bvS[0, h * 65:h * 65 + 64] = bv[sl][h * 64:(h + 1) * 64]
            bvS[0, h * 65 + 64] = 1.0
        in_maps.append({
            "HT": np.ascontiguousarray(H[b].T).astype(bf),
            "WqR": to_sbuf_layout_m(Wq[sl].T).astype(bf),
            "WkR": to_sbuf_layout_m(Wk[sl].T).astype(bf),
            "WvR": to_sbuf_layout(WvS).astype(bf),
            "WoS": np.ascontiguousarray(Wo[:, sl].T).astype(bf),
            "bqc": np.ascontiguousarray(bq[sl].reshape(2, 128).T.astype(np.float32)),
            "bkc": np.ascontiguousarray(bk[sl].reshape(2, 128).T.astype(np.float32)),
            "bvS": bvS,
            "kpm": np.ascontiguousarray(keep[b].reshape(NT, 128).T),
        })
    return in_maps


def kernel(H, key_padding_mask, Wq, bq, Wk, bk, Wv, bv, Wo, bo, _run_kwargs=None):
    H = np.asarray(H, dtype=np.float32)
    Wq = np.asarray(Wq, dtype=np.float32)
    Wk = np.asarray(Wk, dtype=np.float32)
    Wv = np.asarray(Wv, dtype=np.float32)
    Wo = np.asarray(Wo, dtype=np.float32)
    bq = np.asarray(bq, dtype=np.float32)
    bk = np.asarray(bk, dtype=np.float32)
    bv = np.asarray(bv, dtype=np.float32)
    bo = np.asarray(bo, dtype=np.float32)

    nc = build()
    in_maps = _prep_core_inputs(H, key_padding_mask, Wq, bq, Wk, bk, Wv, bv, Wo, bo)
    res = bass_utils.run_bass_kernel_spmd(
        nc, in_maps, core_ids=list(range(8)), **(_run_kwargs or {})
    )
    out = np.zeros((B, T, D), dtype=np.float32)
    for c in range(8):
        out[c // 4] += res.results[c]["O"].astype(np.float32)
    out += bo
    if _run_kwargs:
        kernel.last_result = res
    return out


# revision 41
# speedup vs baseline: 1.0323x; 1.0148x over previous
"""Multi-head self-attention (B=2, T=2048, D=1024, 16 heads) on 8 TRN2 cores.

Sharding: core c = (b, g) with b = c // 4 (batch), g = c % 4 (head group of 4).
Each core computes q/k/v projections for its 4 heads, causal softmax
attention, and a partial output projection (its 256 columns of the
concat-head dim against Wo). Host sums the 4 partials per batch and adds bo.

v3: same fully-interleaved pass as v2 (attention chunk pipeline with
projection / output-projection fillers), plus:
  - token-sliced HT input DMA across 3 queues so the bootstrap
    projections start at ~25% of the HT fill instead of 100%;
  - compact vt tiles [128, 260] (4 heads x 65: 64 v dims + a ones
    column that makes AV emit softmax denominators) -- no zero padding,
    no gpsimd memsets;
  - the final block's softmax normalization uses an ACT-table
    reciprocal on the PSUM denominator row + a PE ones-matmul
    partition-broadcast instead of the DRAM-bounce + gpsimd path,
    cutting the serial tail;
  - per-half output tiles so the two DMA halves of each O row chunk
    are independent.

Per-core layout:
  qT/kT [128, 2048] bf16: rows = 2 heads x 64 dims, cols = tokens.
  vt[t] [128, 260] bf16: rows = 128 key tokens of chunk t, cols =
     4 heads x 65 (64 v dims + a 1.0 column).
  attT [128, 2048] bf16 per head pair: normalized A^T V rows.
  O [2048, 1024] bf16 partial output, summed on host in f32.
"""

import ml_dtypes
import numpy as np

import concourse.bass as bass
import concourse.tile as tile
from concourse import bacc, mybir
from concourse import bass_utils
from contextlib import ExitStack

F32 = mybir.dt.float32
BF16 = mybir.dt.bfloat16
ATT = BF16
AF = mybir.ActivationFunctionType
OP = mybir.AluOpType

B, T, D = 2, 2048, 1024
NH, DH = 16, 64
HPC = 4            # heads per core
GD = HPC * DH      # 256, group dim
GV = HPC * (DH + 1)  # 260, v tile width (compact, 65 per head)
NKD = D // 128     # 8 K-chunks for projections
NT = T // 128      # 16 token chunks
NJ = T // 512      # 4 query blocks
N_WARM = 12        # HAM clock-ramp warmup matmuls

_NC_CACHE = {}


def build():
    if "nc" in _NC_CACHE:
        return _NC_CACHE["nc"]
    nc = bacc.Bacc("TRN2", target_bir_lowering=False, debug=False, num_devices=8)

    HT = nc.dram_tensor("HT", [D, T], BF16, kind="ExternalInput").ap()
    # weights pre-permuted on the host to the SBUF layout (m-major, then
    # k-chunk-major) so input DMAs are plain streams, not slow gathers --
    # and the m=1 half can be deferred past the bootstrap-critical bytes
    WqR = nc.dram_tensor("WqR", [128, NKD * GD], BF16, kind="ExternalInput").ap()
    WkR = nc.dram_tensor("WkR", [128, NKD * GD], BF16, kind="ExternalInput").ap()
    WvR = nc.dram_tensor("WvR", [128, NKD * GV], BF16, kind="ExternalInput").ap()
    WoS = nc.dram_tensor("WoS", [GD, D], BF16, kind="ExternalInput").ap()
    bqc = nc.dram_tensor("bqc", [128, 2], F32, kind="ExternalInput").ap()
    bkc = nc.dram_tensor("bkc", [128, 2], F32, kind="ExternalInput").ap()
    bvS = nc.dram_tensor("bvS", [1, GV], F32, kind="ExternalInput").ap()
    kpm = nc.dram_tensor("kpm", [128, NT], F32, kind="ExternalInput").ap()
    O = nc.dram_tensor("O", [T, D], BF16, kind="ExternalOutput").ap()
    zd = nc.dram_tensor("zd", [8, 1024], F32, kind="Internal").ap()
    rd = nc.dram_tensor("rd", [8, 1024], F32, kind="Internal").ap()

    with tile.TileContext(nc) as tc, ExitStack() as octx:
        cpool = octx.enter_context(tc.tile_pool(name="const", bufs=1))
        keep = octx.enter_context(tc.tile_pool(name="keep", bufs=1))
        work = octx.enter_context(tc.tile_pool(name="work", bufs=1))
        ps_sc = octx.enter_context(tc.tile_pool(name="ps_sc", bufs=1, space="PSUM"))
        ps_at = octx.enter_context(tc.tile_pool(name="ps_at", bufs=1, space="PSUM"))
        ps_sm = octx.enter_context(tc.tile_pool(name="ps_sm", bufs=1, space="PSUM"))

        # ---- constants ----
        bq_sb = cpool.tile([128, 2], F32, name="bq_sb", tag="bq_sb")
        bk_sb = cpool.tile([128, 2], F32, name="bk_sb", tag="bk_sb")
        bv_sb = cpool.tile([1, GV], F32, name="bv_sb", tag="bv_sb")
        kpm_sb = cpool.tile([128, NT], F32, name="kpm_sb", tag="kpm_sb")

        # ---- long-lived activations ----
        qT = [keep.tile([128, T], ATT, name=f"qT{m}", tag=f"qT{m}") for m in range(2)]
        kT = [keep.tile([128, T], ATT, name=f"kT{m}", tag=f"kT{m}") for m in range(2)]
        vt = [keep.tile([128, GV], ATT, name=f"vt{t}", tag=f"vt{t}") for t in range(NT)]
        attT = [keep.tile([128, T], ATT, name=f"attT{m}", tag=f"attT{m}") for m in range(2)]
        wo_r = [keep.tile([128, D], ATT, name=f"wo{i}", tag=f"wo{i}") for i in range(2)]

        # input H^T, token-sliced into four merged tiles (one per 512-token
        # quarter, k-chunk-major inside) so projections can start at ~25%
        # of the HT fill and each filler only depends on one quarter.
        ht_q = [work.tile([128, 4096], BF16, name=f"ht_q{n}", tag=f"ht_q{n}")
                for n in range(4)]

        def ht_blk(k, n):
            # [128, 512] slice of H^T k-chunk covering tokens n*512:(n+1)*512
            return ht_q[n][:, k * 512:(k + 1) * 512]

        def ht_tok(k, t):
            # [128, 128] slice covering token chunk t
            n, o = divmod(t, 4)
            return ht_blk(k, n)[:, o * 128:(o + 1) * 128]

        wq_r = work.tile([128, NKD * GD], BF16, name="wq_r", tag="wq_r")
        wk_r = work.tile([128, NKD * GD], BF16, name="wk_r", tag="wk_r")
        wv_r = work.tile([128, NKD * GV], BF16, name="wv_r", tag="wv_r")

        # ---- input DMA ----
        # vector queue: warmup memset only
        warm = cpool.tile([128, 512], BF16, name="warm", tag="warm")
        nc.vector.memset(warm[:], 0.0)
        # ALL input transfers go on ONE queue in strict need order: with two
        # queues the rings race into not-yet-needed bytes and starve the
        # bootstrap-critical set (everything shares the ~358 GB/s HBM pipe).
        # Need order: m=0 weight halves + wv + HT first quarter (~2 MB,
        # feeds the bootstrap), HT second quarter, HT back half, m=1 weight
        # halves, Wo.
        HW = NKD * 128  # 1024: one m-half of wq/wk

        def ht_dma(dst, rows, cols, nk):
            nc.scalar.dma_start(
                dst.rearrange("p (k t) -> p k t", k=nk),
                HT[rows[0]:rows[1], cols[0]:cols[1]]
                .rearrange("(k p) t -> p k t", k=nk),
            )

        # tiny per-core constants lead the stream (gpsimd-issued DMAs would
        # land on a software-DGE ring behind the whole input stream and
        # stall bvrow -> every v-projection until ~23us)
        nc.scalar.dma_start(bv_sb[:], bvS[:])
        nc.scalar.dma_start(bq_sb[:], bqc[:])
        nc.scalar.dma_start(bk_sb[:], bkc[:])
        nc.scalar.dma_start(kpm_sb[:], kpm[:])
        nc.scalar.dma_start(wq_r[:, 0:HW], WqR[:, 0:HW])
        ht_dma(ht_q[0][:, 0:2048], (0, 512), (0, 512), 4)
        ht_dma(ht_q[0][:, 2048:4096], (512, 1024), (0, 512), 4)
        nc.scalar.dma_start(wk_r[:, 0:HW], WkR[:, 0:HW])
        nc.scalar.dma_start(wv_r[:], WvR[:])
        ht_dma(ht_q[1][:, :], (0, 1024), (512, 1024), 8)
        ht_dma(ht_q[2][:, 0:2048], (0, 512), (1024, 1536), 4)
        ht_dma(ht_q[2][:, 2048:4096], (512, 1024), (1024, 1536), 4)
        ht_dma(ht_q[3][:, 0:2048], (0, 512), (1536, 2048), 4)
        ht_dma(ht_q[3][:, 2048:4096], (512, 1024), (1536, 2048), 4)
        nc.scalar.dma_start(wq_r[:, HW:2 * HW], WqR[:, HW:2 * HW])
        nc.scalar.dma_start(wk_r[:, HW:2 * HW], WkR[:, HW:2 * HW])
        for i in range(2):
            nc.scalar.dma_start(wo_r[i][:], WoS[i * 128:(i + 1) * 128, :])
        # gpsimd queue: masks/constants, bv broadcast (stays off the HBM
        # critical path; later it only issues output DMAs)
        tri = cpool.tile([128, 128], ATT, name="tri", tag="tri")
        nc.gpsimd.memset(tri[:], 1.0)
        nc.gpsimd.affine_select(
            out=tri[:], in_=tri[:], compare_op=OP.is_ge, fill=0.0,
            base=0, pattern=[[1, 128]], channel_multiplier=-1,
        )
        # bv broadcast across partitions: [128, GV]
        bvrow = cpool.tile([128, GV], F32, name="bvrow", tag="bvrow")
        nc.gpsimd.partition_broadcast(bvrow[:], bv_sb[:])

        # HAM warmup: keep the PE array busy during the input-DMA fill so
        # the clock gate reaches K=8/8 before real matmuls start (zero
        # data, the results are never read).
        for _ in range(N_WARM):
            wp = ps_sm.tile([128, 512], F32, name="wp", tag="smps", bufs=2)
            nc.tensor.matmul(wp[:], warm[:, 0:128], warm[:], start=True, stop=True)

        # ---- filler work units (PE work injected between attention chunks) ----
        def proj_qk(w_r, dest, bias_sb, m, n, c0=0, c1=512):
            # dest[m][:, n*512+c0 : n*512+c1] = sum_k W_k[:, m-block].T @ ht_k + bias
            w = c1 - c0
            ps = ps_sm.tile([128, 512], F32, name="pp", tag="smps", bufs=2)
            for k in range(NKD):
                nc.tensor.matmul(
                    ps[:, 0:w],
                    w_r[:, (m * NKD + k) * 128:(m * NKD + k) * 128 + 128],
                    ht_blk(k, n)[:, c0:c1],
                    start=(k == 0), stop=(k == NKD - 1),
                )
            # bias add folded into the PSUM->SBUF copy (DVE, per-partition scalar)
            nc.vector.tensor_scalar_add(
                dest[m][:, n * 512 + c0:n * 512 + c1], ps[:, 0:w],
                bias_sb[:, m:m + 1]
            )

        def proj_v(t):
            # vt[t] per-head blocks = (sum_k ht_k_t.T @ WvS_k + bv) * kpm
            vp = ps_sm.tile([128, 512], F32, name="vp", tag="smps", bufs=2)
            for k in range(NKD):
                nc.tensor.matmul(
                    vp[:, 0:GV],
                    ht_tok(k, t),
                    wv_r[:, k * GV:(k + 1) * GV],
                    start=(k == 0), stop=(k == NKD - 1),
                )
            nc.vector.tensor_tensor(
                vt[t][:], vp[:, 0:GV], bvrow[:], op=OP.add,
            )
            nc.vector.tensor_scalar_mul(vt[t][:], vt[t][:], kpm_sb[:, t:t + 1])

        def out_half(t, n):
            # O[t-chunk, n-half] = sum_hp attT[hp][:, t-chunk].T @ WoS[hp][:, n-half]
            ot = work.tile([128, 512], BF16, name="ot", tag="ot", bufs=4)
            op = ps_sm.tile([128, 512], F32, name="op", tag="smps", bufs=2)
            for hp in range(2):
                nc.tensor.matmul(
                    op[:],
                    attT[hp][:, t * 128:(t + 1) * 128],
                    wo_r[hp][:, n * 512:(n + 1) * 512],
                    start=(hp == 0), stop=(hp == 1),
                )
            # PSUM->SBUF copy on the scalar engine (Identity shares the Exp
            # ACT table): frees the DVE, which runs the normalize multiplies.
            # Output DMA issues from the gpsimd queue (idle mid-kernel).
            nc.scalar.activation(ot[:], op[:], AF.Identity)
            nc.gpsimd.dma_start(O[t * 128:(t + 1) * 128, n * 512:(n + 1) * 512],
                                ot[:])

        fillers = []

        def pop_filler():
            if fillers:
                fillers.pop(0)()

        # ---- attention block machinery ----
        def normalize(hp, J, at):
            # zau: unnormalized A^T V rows (0:64) + denominator row (64)
            zaus = []
            for hh in range(2):
                zau = work.tile([65, 512], F32, name="zau", tag="zau", bufs=4)
                nc.vector.tensor_copy(zau[:], at[hh][0:65, :])
                zaus.append(zau)
            bi = hp * 4 + J
            # exact reciprocal on a partition-packed [128, 8] tile: bounce the
            # two denominator rows through DRAM (engines cannot cross
            # partitions; DMA can). 8 elem/lane keeps the iterative divide
            # at ~130 ns instead of 4.3 us on a [1, 512] row.
            for hh in range(2):
                nc.sync.dma_start(zd[bi:bi + 1, hh * 512:(hh + 1) * 512],
                                    zaus[hh][64:65, :])
            zp = work.tile([128, 8], F32, name="zp", tag="zp", bufs=2)
            nc.sync.dma_start(
                zp[:], zd[bi:bi + 1, :].rearrange("p (a b) -> (p a) b", b=8)
            )
            rp = work.tile([128, 8], F32, name="rp", tag="rp", bufs=2)
            nc.vector.reciprocal(rp[:], zp[:])
            nc.sync.dma_start(
                rd[bi:bi + 1, :].rearrange("p (a b) -> (p a) b", b=8), rp[:]
            )
            for hh in range(2):
                zau = zaus[hh]
                # partition-broadcast 1/z straight out of DRAM with a
                # 0-stride DMA read (frees gpsimd, one fewer serial hop)
                rb = work.tile([64, 512], F32, name="rb", tag="rb", bufs=4)
                nc.sync.dma_start(
                    rb[:],
                    rd[bi:bi + 1, hh * 512:(hh + 1) * 512]
                    .squeeze(0).partition_broadcast(64),
                )
                nc.vector.tensor_tensor(
                    attT[hp][hh * 64:(hh + 1) * 64, J * 512:(J + 1) * 512],
                    zau[0:64, :],
                    rb[:],
                    op=OP.mult,
                )

        def block(J, hp):
            at = [
                ps_at.tile([128, 512], F32, name=f"at{hh}", tag="av", bufs=2)
                for hh in range(2)
            ]
            # diagonal chunk first (full width, opens PSUM accumulation),
            # then off-diagonals, then narrow diagonals.
            kcs = [4 * J] + list(range(4 * J)) + [4 * J + i for i in range(1, 4)]

            def issue_sc_exp(kc):
                off = max(0, 128 * (kc - 4 * J))
                w = 512 - off
                sc = ps_sc.tile([128, 1024], F32, name="sc", tag="sc", bufs=2)
                for hh in range(2):
                    nc.tensor.matmul(
                        sc[:, hh * 512:hh * 512 + w],
                        kT[hp][hh * 64:(hh + 1) * 64, kc * 128:(kc + 1) * 128],
                        qT[hp][hh * 64:(hh + 1) * 64, J * 512 + off:(J + 1) * 512],
                        start=True, stop=True,
                        tile_position=(hh * 64, 0),
                    )
                ex = work.tile([128, 1024], ATT, name="ex", tag="ex", bufs=8)
                nc.scalar.activation(
                    ex[:].rearrange("p (h c) -> p h c", c=512)[:, :, 0:w],
                    sc[:].rearrange("p (h c) -> p h c", c=512)[:, :, 0:w],
                    AF.Exp, scale=0.125,
                )
                if off or kc == 4 * J:
                    for hh in range(2):
                        nc.vector.tensor_tensor(
                            ex[:, hh * 512:hh * 512 + 128],
                            ex[:, hh * 512:hh * 512 + 128],
                            tri[:],
                            op=OP.mult,
                        )
                return ex

            def issue_av(kc, ex, first, last):
                off = max(0, 128 * (kc - 4 * J))
                w = 512 - off
                for hh in range(2):
                    h = 2 * hp + hh
                    nc.tensor.matmul(
                        at[hh][0:65, off:512],
                        vt[kc][:, h * 65:(h + 1) * 65],
                        ex[:, hh * 512:hh * 512 + w],
                        start=first, stop=last,
                    )

            prev = None
            for ti, kc in enumerate(kcs):
                ex = issue_sc_exp(kc)
                pop_filler()
                if prev is not None:
                    issue_av(prev[0], prev[1], first=(prev[2] == 0), last=False)
                prev = (kc, ex, ti)
            issue_av(prev[0], prev[1], first=(prev[2] == 0), last=True)
            normalize(hp, J, at)

        # ---- bootstrap projections for the first diagonal chunk ----
        # minimal set for the first score/AV chunk: full q block, the first
        # 128 key columns, vt[0]; everything else becomes fillers so the PE
        # never idles (idling drops the HAM clock gate to K=4).
        proj_qk(wq_r, qT, bq_sb, 0, 0)
        proj_qk(wk_r, kT, bk_sb, 0, 0, 0, 128)
        proj_v(0)

        def qk(hp_, n_):
            fillers.append(lambda: proj_qk(wq_r, qT, bq_sb, hp_, n_))
            fillers.append(lambda: proj_qk(wk_r, kT, bk_sb, hp_, n_))

        # filler schedule, matched to each block's pop budget (block(J,hp)
        # pops 4J+4 fillers; two extra boundary pops follow block(0,0)) and
        # to the token-sliced HT DMA arrival order. Each proj must be popped
        # no later than its first consumer chunk in the block pipeline.
        fillers.append(lambda: proj_qk(wk_r, kT, bk_sb, 0, 0, 128, 512))
        for t in (1, 2, 3):
            fillers.append(lambda t=t: proj_v(t))
        qk(0, 1)                                     # boundary pops
        for t in (4, 5, 6, 7):                       # block(1,0) pops
            fillers.append(lambda t=t: proj_v(t))
        qk(0, 2)
        for t in (8, 9):
            fillers.append(lambda t=t: proj_v(t))
        for t in (10, 11):                           # block(2,0) pops
            fillers.append(lambda t=t: proj_v(t))
        qk(0, 3)
        for n in reversed(range(4)):
            qk(1, n)
        for t in (12, 13, 14, 15):                   # block(3,0) pops
            fillers.append(lambda t=t: proj_v(t))

        # ---- main pass ----
        # hp1 runs J descending: the big J=3 block comes first (making its
        # output-projection fillers available early) and the small J=0 block
        # lands last, shortening the final normalize->out tail.
        for hp, Js in ((0, range(NJ)), (1, reversed(range(NJ)))):
            for J in Js:
                block(J, hp)
                if hp == 0 and J == 0:
                    pop_filler()
                    pop_filler()
                if hp == 1:
                    # attT for both head pairs at J is now final
                    for t in range(4 * J, 4 * J + 4):
                        for n in range(2):
                            fillers.append(lambda t=t, n=n: out_half(t, n))
        while fillers:
            fillers.pop(0)()

    nc.compile()
    _NC_CACHE["nc"] = nc
    return nc


def _prep_core_inputs(H, key_padding_mask, Wq, bq, Wk, bk, Wv, bv, Wo, bo):
    keep = 1.0 - np.asarray(key_padding_mask, dtype=np.float32)  # [B, T]
    bf = ml_dtypes.bfloat16
    in_maps = []
    def to_sbuf_layout(WT):
        # [D, G] -> [128, NKD*G]: row p holds k-chunk-major slices
        G = WT.shape[1]
        return np.ascontiguousarray(
            WT.reshape(NKD, 128, G).transpose(1, 0, 2).reshape(128, NKD * G))

    def to_sbuf_layout_m(WT):
        # [D, 256] -> [128, 2*NKD*128]: m-half-major, then k-chunk-major
        return np.ascontiguousarray(
            WT.reshape(NKD, 128, 2, 128).transpose(1, 2, 0, 3)
            .reshape(128, 2 * NKD * 128))

    for c in range(8):
        b, g = divmod(c, 4)
        sl = slice(g * GD, (g + 1) * GD)
        WvT = Wv[sl].T  # [D, GD]
        WvS = np.zeros((D, GV), dtype=np.float32)
        bvS = np.zeros((1, GV), dtype=np.float32)
        for h in range(HPC):
            WvS[:, h * 65:h * 65 + 64] = WvT[:, h * 64:(h + 1) * 64]
            bvS[0, h * 65:h * 65 + 64] = bv[sl][h * 64:(h + 1) * 64]
            bvS[0, h * 65 + 64] = 1.0
        in_maps.append({
            "HT": np.ascontiguousarray(H[b].T).astype(bf),
            "WqR": to_sbuf_layout_m(Wq[sl].T).astype(bf),
            "WkR": to_sbuf_layout_m(Wk[sl].T).astype(bf),
            "WvR": to_sbuf_layout(WvS).astype(bf),
            "WoS": np.ascontiguousarray(Wo[:, sl].T).astype(bf),
            "bqc": np.ascontiguousarray(bq[sl].reshape(2, 128).T.astype(np.float32)),
            "bkc": np.ascontiguousarray(bk[sl].reshape(2, 128).T.astype(np.float32)),
            "bvS": bvS,
            "kpm": np.ascontiguousarray(keep[b].reshape(NT, 128).T),
        })
    return in_maps


def kernel(H, key_padding_mask, Wq, bq, Wk, bk, Wv, bv, Wo, bo, _run_kwargs=None):
    H = np.asarray(H, dtype=np.float32)
    Wq = np.asarray(Wq, dtype=np.float32)
    Wk = np.asarray(Wk, dtype=np.float32)
    Wv = np.asarray(Wv, dtype=np.float32)
    Wo = np.asarray(Wo, dtype=np.float32)
    bq = np.asarray(bq, dtype=np.float32)
    bk = np.asarray(bk, dtype=np.float32)
    bv = np.asarray(bv, dtype=np.float32)
    bo = np.asarray(bo, dtype=np.float32)

    nc = build()
    in_maps = _prep_core_inputs(H, key_padding_mask, Wq, bq, Wk, bk, Wv, bv, Wo, bo)
    res = bass_utils.run_bass_kernel_spmd(
        nc, in_maps, core_ids=list(range(8)), **(_run_kwargs or {})
    )
    out = np.zeros((B, T, D), dtype=np.float32)
    for c in range(8):
        out[c // 4] += res.results[c]["O"].astype(np.float32)
    out += bo
    if _run_kwargs:
        kernel.last_result = res
    return out
